# revision 43
# baseline (speedup 1.0000x reference)
"""AllPairsSimilarity Trainium2 kernel (8 NeuronCores, data-parallel over NQ).

Per query image q (C=640, HW=1024) against a 5-shot support set:
    proto = sum_shots(support)              # (C, M)
    shat  = fp8(proto / ||proto||_col)      # column-normalized, fp8 e4m3
    sim   = q8.T @ shat                     # (HW, M) fp8 DoubleRow matmuls
    score = mean_n max_m sim[n, m] / ||q_n||
The per-n query norm is applied after the max (constant over m); its
squares run on ACT, the summand adds on the otherwise-idle GPSIMD, and the
cross-partition sum is a tiny ones-matmul issued one image late so the PE
never waits on it.

Matmul: contraction over 640 channels = 2 fp8 DoubleRow matmuls (K=256)
+ 1 normal fp8 matmul (K=128) per psum half -> 1670 cyc/tile vs 2560 bf16.

Sharding: query images split 16 per core; support replicated.
"""
import os
import sys

for _p in ("/opt/trn_rl_repo",):
    if _p not in sys.path and os.path.isdir(_p):
        sys.path.insert(0, _p)

import numpy as np

import concourse.bass as bass  # noqa: E402
import concourse.tile as tile  # noqa: E402
from concourse import bacc, bass_isa, mybir  # noqa: E402
from concourse import bass_utils  # noqa: E402
from concourse.tile_rust import add_dep_helper  # noqa: E402

F32 = mybir.dt.float32
F32R = mybir.dt.float32r
FP8 = mybir.dt.float8e4
BF16 = mybir.dt.bfloat16
AX_X = mybir.AxisListType.X
MULT = mybir.AluOpType.mult
DR = mybir.MatmulPerfMode.DoubleRow

NQ_SH = 16   # query images per core
CB = 5       # 640 = 5 x 128 channel blocks
N = 1024     # query patches per image (HW)
M = 1024     # support patches
NCORES = 8


def build_bass(n_sq_act=5, simp_bufs=4, qstg_bufs=3, qld_bufs=3,
               n_gp_adds=4, no_dr=False, no_ar=False, n_sim=NQ_SH):
    nc = bacc.Bacc("TRN2", target_bir_lowering=False, debug=False)
    q_d = nc.declare_dram_parameter("q", [NQ_SH, CB * 128, N], F32, isOutput=False)
    s_d = nc.declare_dram_parameter("s", [5, CB * 128, M], F32, isOutput=False)
    out_d = nc.declare_dram_parameter("out", [1, NQ_SH], F32, isOutput=True)

    with tile.TileContext(nc) as tc:
        with tc.tile_pool(name="const", bufs=1) as const_p, \
             tc.tile_pool(name="sp8", bufs=1) as sp8_p, \
             tc.tile_pool(name="norms", bufs=1) as norm_p, \
             tc.tile_pool(name="maxc", bufs=1) as maxc_p, \
             tc.tile_pool(name="sload", bufs=2) as slp, \
             tc.tile_pool(name="proto", bufs=1) as prp, \
             tc.tile_pool(name="psq", bufs=2) as psqp, \
             tc.tile_pool(name="pmisc", bufs=2) as miscp, \
             tc.tile_pool(name="qstg", bufs=qstg_bufs) as qstgp, \
             tc.tile_pool(name="qld", bufs=qld_bufs) as qlp, \
             tc.tile_pool(name="qsq", bufs=2) as qsqp, \
             tc.tile_pool(name="qsum", bufs=3) as qsmp, \
             tc.tile_pool(name="stash", bufs=2) as stp:

            ones_f = const_p.tile([128, 1], F32, tag="ones_f")
            nc.vector.memset(ones_f[:], 1.0)
            ones_b = const_p.tile([128, 1], BF16, tag="ones_b")
            nc.vector.memset(ones_b[:], 1.0)
            ones_k1 = const_p.tile([1, 128], F32R, tag="ones_k1")
            nc.sync.dma_start(ones_k1[:], ones_f[:].bitcast(F32R))

            # column layout: [j, img*8 + nb] = value for patch n = nb*128+j
            norms2cols = norm_p.tile([128, 128], F32, tag="n2")
            maxcols = maxc_p.tile([128, 128], F32, tag="mc")

            # fp8 shat: two DoubleRow pair tiles + leftover block
            sp8 = [sp8_p.tile([128, 2, M], FP8, tag=f"sp8_{p}", name=f"sp8_{p}")
                   for p in range(2)]
            s84 = sp8_p.tile([128, M], FP8, tag="s84")

            # prototype-norm psum lives only through the support prologue;
            # closed before the epilogue psum pool opens (8-bank budget)
            pnps_ctx = tc.tile_pool(name="pnps", bufs=1, space="PSUM")
            pnps = pnps_ctx.__enter__()
            pn_ps = pnps.tile([1, M], F32, tag="pn")

            protos = [None] * CB

            # -------- support block per (channel-block, psum-half) --------
            s_sh = [None] * CB
            last_sdma = [None]

            def s_block(cb, h):
                hs = slice(h * 512, (h + 1) * 512)
                if h == 0:
                    s_sh[cb] = [slp.tile([128, M], F32, tag=f"sl{sh}",
                                         name=f"sl{sh}") for sh in range(5)]
                    protos[cb] = prp.tile([128, M], BF16, tag=f"pr{cb}",
                                          name=f"pr{cb}")
                sh_t = s_sh[cb]
                for sh in range(5):
                    di = nc.sync.dma_start(sh_t[sh][:, hs],
                                           s_d[sh, cb * 128:(cb + 1) * 128, hs])
                    last_sdma[0] = di
                nc.vector.tensor_add(sh_t[0][:, hs], sh_t[0][:, hs],
                                     sh_t[1][:, hs])
                nc.vector.tensor_add(sh_t[2][:, hs], sh_t[2][:, hs],
                                     sh_t[3][:, hs])
                nc.vector.tensor_add(sh_t[0][:, hs], sh_t[0][:, hs],
                                     sh_t[2][:, hs])
                pr = protos[cb]
                nc.vector.tensor_add(pr[:, hs], sh_t[0][:, hs], sh_t[4][:, hs])
                sq = psqp.tile([128, M], BF16, tag="psq")
                nc.scalar.square(sq[:, hs], pr[:, hs])
                nc.tensor.matmul(pn_ps[:, hs], ones_b[:], sq[:, hs],
                                 start=(cb == 0), stop=(cb == CB - 1))

            # ---------------- query load / norm blocks ----------------
            qtiles = [None] * NQ_SH   # (qp0, qp1, q84) fp8 tiles per image

            def q_load(img):
                qp = [qlp.tile([128, 2, N], FP8, tag=f"qp{p}", name=f"qp{p}")
                      for p in range(2)]
                q4 = qlp.tile([128, N], FP8, tag="q4")
                for cb in range(CB):
                    st = qstgp.tile([128, N], F32, tag=f"qf{cb}")
                    di = nc.sync.dma_start(st[:],
                                           q_d[img, cb * 128:(cb + 1) * 128, :])
                    if last_sdma[0] is not None:
                        # don't let later query loads steal DMA bandwidth
                        # from the support prologue (shat gates all sims)
                        add_dep_helper(di.ins, last_sdma[0].ins, sync=True,
                                       reason="yield DMA bw to support load")
                    dst = qp[cb // 2][:, cb % 2, :] if cb < 4 else q4[:]
                    nc.scalar.copy(dst, st[:])
                qtiles[img] = (qp[0], qp[1], q4)

            simp_box = [None]
            qsums = [None] * NQ_SH
            ADD = mybir.AluOpType.add

            def q_sum(img):
                qp0, qp1, q4 = qtiles[img]
                slots = [qp0[:, 0, :], qp0[:, 1, :], qp1[:, 0, :], qp1[:, 1, :],
                         q4[:]]
                sqs = []
                for cb in range(CB):
                    sq = qsqp.tile([128, N], BF16, tag=f"qsq{cb}", name=f"qsq{cb}")
                    if cb < n_sq_act:
                        nc.scalar.square(sq[:], slots[cb])
                    else:
                        nc.vector.tensor_mul(sq[:], slots[cb], slots[cb])
                    sqs.append(sq)
                qsum = qsmp.tile([128, N], BF16, tag="qsum")
                adds = [(sqs[0], sqs[0], sqs[1]), (sqs[2], sqs[2], sqs[3]),
                        (sqs[0], sqs[0], sqs[2]), (qsum, sqs[0], sqs[4])]
                for k, (o, a, b) in enumerate(adds):
                    if k < n_gp_adds:
                        nc.gpsimd.tensor_tensor(o[:], a[:], b[:], ADD)
                    else:
                        nc.vector.tensor_add(o[:], a[:], b[:])
                qsums[img] = qsum

            def q_fin(img):
                qsum = qsums[img]
                qn_ps = simp_box[0].tile([1, M], F32, tag="ps", name="qn_ps")
                for h in range(2):
                    nc.tensor.matmul(qn_ps[:, h * 512:(h + 1) * 512], ones_b[:],
                                     qsum[:, h * 512:(h + 1) * 512],
                                     start=True, stop=True)
                # stash the (1, 1024) norm row permuted to j-major order so one
                # contiguous DMA scatters it into column layout
                qn_sb = stp.tile([1, N], F32, tag="stash")
                nc.vector.tensor_scalar_add(
                    qn_sb[:, :].rearrange("one (j nb) -> one j nb", j=128, nb=8),
                    qn_ps[0:1, :].rearrange("one (nb j) -> one j nb", nb=8, j=128),
                    0.0)
                nc.sync.dma_start(norms2cols[:, img * 8:(img + 1) * 8], qn_sb[:, :])

            # ---------------- sim matmuls + row max ----------------
            def sim_block(img):
                if img >= n_sim:
                    return
                qp0, qp1, q4 = qtiles[img]
                for nb in range(8):
                    ps = simp_box[0].tile([128, M], F32, tag="ps", name="ps")
                    if no_dr:
                        qps = [qp0, qp0, qp1, qp1, None]
                        for p in range(5):
                            for h in range(2):
                                hs = slice(h * 512, (h + 1) * 512)
                                ns = slice(nb * 128, (nb + 1) * 128)
                                if p < 4:
                                    l5 = qps[p][:, p % 2, ns]
                                    r5 = sp8[p // 2][:, p % 2, hs]
                                else:
                                    l5 = q4[:, ns]
                                    r5 = s84[:, hs]
                                nc.tensor.matmul(
                                    ps[:, hs], l5, r5,
                                    start=(p == 0), stop=(p == 4))
                    else:
                        lhs = [qp0[:, :, nb * 128:(nb + 1) * 128],
                               qp1[:, :, nb * 128:(nb + 1) * 128],
                               q4[:, nb * 128:(nb + 1) * 128]]
                        for p in range(3):
                            for h in range(2):
                                r = sp8[p][:, :, h * 512:(h + 1) * 512] if p < 2 \
                                    else s84[:, h * 512:(h + 1) * 512]
                                nc.tensor.matmul(
                                    ps[:, h * 512:(h + 1) * 512], lhs[p], r,
                                    start=(p == 0), stop=(p == 2),
                                    perf_mode=DR if p < 2 else None)
                    col = img * 8 + nb
                    nc.vector.reduce_max(maxcols[:, col:col + 1], ps[:, :],
                                         axis=AX_X)

            # prototype norm chain (per psum-half) -> shat in fp8.
            # Broadcast pnorm to all partitions via K=1 matmul FIRST, then
            # take the reciprocal full-width on DVE — avoids the slow
            # 1-partition reciprocal and two DMA round-trip latency hops.
            pnorm = miscp.tile([1, M], F32R, tag="pn")
            bc_ps = pnps.tile([128, 512], F32, tag="bc", name="bc_ps")
            bc_sb = miscp.tile([128, M], BF16, tag="bc_sb")

            def chain_h(h):
                hs = slice(h * 512, (h + 1) * 512)
                nc.scalar.sqrt(pnorm[:, hs], pn_ps[:, hs])
                nc.tensor.matmul(bc_ps[:, :], ones_k1[:], pnorm[:, hs],
                                 start=True, stop=True)
                with nc.allow_low_precision(reason="reciprocal, bf16 scale"):
                    nc.vector.reciprocal(bc_sb[:, hs], bc_ps[:, :])
                for cb in range(CB):
                    dst = sp8[cb // 2][:, cb % 2, hs] if cb < 4 else s84[:, hs]
                    nc.vector.tensor_mul(dst, protos[cb][:, hs], bc_sb[:, hs])

            # ---------------- issue order ----------------
            for cb in range(4):
                s_block(cb, 0)
                s_block(cb, 1)
            # issue both s4 halves before the chain: chain_h(0)'s dependent
            # pn8/pinv DMA triggers would otherwise block s4h1's DMA triggers
            # in the sync-engine queue (head-of-line)
            s_block(4, 0)
            s_block(4, 1)
            chain_h(0)
            chain_h(1)
            # q0 is issued after the chain: its gated DMAs land during the
            # pnorm chain, and its ACT conversions must not head-of-line
            # block the chain's sqrt/squares in the scalar-engine queue
            q_load(0)
            pnps_ctx.__exit__(None, None, None)
            # sim psum ring opens only after the prologue psum frees its banks
            simp_ctx = tc.tile_pool(name="simps", bufs=simp_bufs, space="PSUM")
            simp_box[0] = simp_ctx.__enter__()

            q_load(1)
            q_sum(0)
            sim_block(0)
            for img in range(1, NQ_SH):
                q_sum(img)
                if img + 1 < NQ_SH:
                    q_load(img + 1)   # staging triggers ahead of the stash DMA
                q_fin(img - 1)
                if img == NQ_SH - 1:
                    q_fin(img)   # before the last sims so the tail is max-drain only
                sim_block(img)
            simp_ctx.__exit__(None, None, None)

            # ---------------- epilogue: scores ----------------
            with tc.tile_pool(name="ep", bufs=1) as ep, \
                 tc.tile_pool(name="eps", bufs=1, space="PSUM") as epp:
                qn = ep.tile([128, 128], F32, tag="qn")
                nc.scalar.sqrt(qn[:], norms2cols[:])
                qinv = ep.tile([128, 128], F32, tag="qinv")
                nc.vector.reciprocal(qinv[:], qn[:])
                scaled = ep.tile([128, 128], F32, tag="scaled")
                nc.vector.scalar_tensor_tensor(
                    out=scaled[:], in0=maxcols[:], scalar=1.0 / float(N),
                    in1=qinv[:], op0=MULT, op1=MULT)
                fin_ps = epp.tile([1, 128], F32, tag="fin")
                nc.tensor.matmul(fin_ps[:], ones_f[:], scaled[:],
                                 start=True, stop=True)
                fin_sb = ep.tile([1, 128], F32, tag="finsb")
                nc.scalar.copy(fin_sb[:], fin_ps[:])
                scores = ep.tile([1, NQ_SH], F32, tag="scores")
                nc.vector.reduce_sum(
                    scores[:],
                    fin_sb[:].rearrange("p (i b) -> p i b", i=16, b=8),
                    axis=AX_X)
                nc.sync.dma_start(out_d[:, :], scores[:])

    nc.compile()
    return nc


_NC_CACHE = None


def _get_nc():
    global _NC_CACHE
    if _NC_CACHE is None:
        _NC_CACHE = build_bass()
    return _NC_CACHE


def kernel(query_features: np.ndarray, support_features: np.ndarray) -> np.ndarray:
    NQ = query_features.shape[0]
    assert NQ == NQ_SH * NCORES
    q = np.ascontiguousarray(
        query_features.reshape(NQ, CB * 128, N).astype(np.float32, copy=False))
    s = np.ascontiguousarray(
        support_features.reshape(5, CB * 128, M).astype(np.float32, copy=False))
    nc = _get_nc()
    in_maps = [
        {"q": q[i * NQ_SH:(i + 1) * NQ_SH], "s": s} for i in range(NCORES)
    ]
    res = bass_utils.run_bass_kernel_spmd(nc, in_maps, core_ids=list(range(NCORES)))
    out = np.concatenate(
        [np.asarray(res.results[i]["out"]).reshape(NQ_SH) for i in range(NCORES)])
    return out.astype(np.float32, copy=False)


# revision 44
# speedup vs baseline: 1.1352x; 1.1352x over previous
"""AllPairsSimilarity Trainium2 kernel (8 NeuronCores, data-parallel over NQ).

Per query image q (C=640, HW=1024) against a 5-shot support set:
    proto = sum_shots(support)              # (C, M)
    shat  = fp8(proto / ||proto||_col)      # column-normalized, fp8 e4m3
    sim   = q8.T @ shat                     # (HW, M) fp8 DoubleRow matmuls
    score = mean_n max_m sim[n, m] / ||q_n||
The per-n query norm is applied after the max (constant over m); its
squares run on ACT, the summand adds on the otherwise-idle GPSIMD, and the
cross-partition sum is a tiny ones-matmul issued one image late so the PE
never waits on it.

Matmul: contraction over 640 channels = 2 fp8 DoubleRow matmuls (K=256)
+ 1 normal fp8 matmul (K=128) per psum half -> 1670 cyc/tile vs 2560 bf16.

Sharding: query images split 16 per core; support replicated.
"""
import os
import sys

for _p in ("/opt/trn_rl_repo",):
    if _p not in sys.path and os.path.isdir(_p):
        sys.path.insert(0, _p)

import numpy as np

import concourse.bass as bass  # noqa: E402
import concourse.tile as tile  # noqa: E402
from concourse import bacc, bass_isa, mybir  # noqa: E402
from concourse import bass_utils  # noqa: E402
from concourse.tile_rust import add_dep_helper  # noqa: E402

F32 = mybir.dt.float32
F32R = mybir.dt.float32r
FP8 = mybir.dt.float8e4
BF16 = mybir.dt.bfloat16
AX_X = mybir.AxisListType.X
MULT = mybir.AluOpType.mult
DR = mybir.MatmulPerfMode.DoubleRow

NQ_SH = 16   # query images per core
CB = 5       # 640 = 5 x 128 channel blocks
N = 1024     # query patches per image (HW)
M = 1024     # support patches
NCORES = 8


def build_bass(n_sq_act=5, simp_bufs=4, qstg_bufs=3, qld_bufs=3,
               n_gp_adds=4, no_dr=False, no_ar=False, n_sim=NQ_SH):
    nc = bacc.Bacc("TRN2", target_bir_lowering=False, debug=False)
    q_d = nc.declare_dram_parameter("q", [NQ_SH, CB * 128, N], F32, isOutput=False)
    s_d = nc.declare_dram_parameter("s", [5, CB * 128, M], F32, isOutput=False)
    out_d = nc.declare_dram_parameter("out", [1, NQ_SH], F32, isOutput=True)

    with tile.TileContext(nc) as tc:
        with tc.tile_pool(name="const", bufs=1) as const_p, \
             tc.tile_pool(name="sp8", bufs=1) as sp8_p, \
             tc.tile_pool(name="norms", bufs=1) as norm_p, \
             tc.tile_pool(name="maxc", bufs=1) as maxc_p, \
             tc.tile_pool(name="sload", bufs=2) as slp, \
             tc.tile_pool(name="proto", bufs=1) as prp, \
             tc.tile_pool(name="psq", bufs=2) as psqp, \
             tc.tile_pool(name="pmisc", bufs=2) as miscp, \
             tc.tile_pool(name="qstg", bufs=qstg_bufs) as qstgp, \
             tc.tile_pool(name="qld", bufs=qld_bufs) as qlp, \
             tc.tile_pool(name="qsq", bufs=2) as qsqp, \
             tc.tile_pool(name="qsum", bufs=3) as qsmp, \
             tc.tile_pool(name="stash", bufs=2) as stp:

            ones_f = const_p.tile([128, 1], F32, tag="ones_f")
            nc.vector.memset(ones_f[:], 1.0)
            ones_b = const_p.tile([128, 1], BF16, tag="ones_b")
            nc.vector.memset(ones_b[:], 1.0)
            ones_k1 = const_p.tile([1, 128], F32R, tag="ones_k1")
            nc.sync.dma_start(ones_k1[:], ones_f[:].bitcast(F32R))

            # column layout: [j, img*8 + nb] = value for patch n = nb*128+j
            norms2cols = norm_p.tile([128, 128], F32, tag="n2")
            maxcols = maxc_p.tile([128, 128], F32, tag="mc")

            # fp8 shat: two DoubleRow pair tiles + leftover block
            sp8 = [sp8_p.tile([128, 2, M], FP8, tag=f"sp8_{p}", name=f"sp8_{p}")
                   for p in range(2)]
            s84 = sp8_p.tile([128, M], FP8, tag="s84")

            # prototype-norm psum lives only through the support prologue;
            # closed before the epilogue psum pool opens (8-bank budget)
            pnps_ctx = tc.tile_pool(name="pnps", bufs=1, space="PSUM")
            pnps = pnps_ctx.__enter__()
            pn_ps = pnps.tile([1, M], F32, tag="pn")

            protos = [None] * CB

            # -------- support block per (channel-block, psum-half) --------
            s_sh = [None] * CB
            last_sdma = [None]

            def s_block(cb, h):
                hs = slice(h * 512, (h + 1) * 512)
                if h == 0:
                    s_sh[cb] = [slp.tile([128, M], F32, tag=f"sl{sh}",
                                         name=f"sl{sh}") for sh in range(5)]
                    protos[cb] = prp.tile([128, M], BF16, tag=f"pr{cb}",
                                          name=f"pr{cb}")
                sh_t = s_sh[cb]
                for sh in range(5):
                    di = nc.sync.dma_start(sh_t[sh][:, hs],
                                           s_d[sh, cb * 128:(cb + 1) * 128, hs])
                    last_sdma[0] = di
                nc.vector.tensor_add(sh_t[0][:, hs], sh_t[0][:, hs],
                                     sh_t[1][:, hs])
                nc.vector.tensor_add(sh_t[2][:, hs], sh_t[2][:, hs],
                                     sh_t[3][:, hs])
                nc.vector.tensor_add(sh_t[0][:, hs], sh_t[0][:, hs],
                                     sh_t[2][:, hs])
                pr = protos[cb]
                nc.vector.tensor_add(pr[:, hs], sh_t[0][:, hs], sh_t[4][:, hs])
                sq = psqp.tile([128, M], BF16, tag="psq")
                nc.scalar.square(sq[:, hs], pr[:, hs])
                nc.tensor.matmul(pn_ps[:, hs], ones_b[:], sq[:, hs],
                                 start=(cb == 0), stop=(cb == CB - 1))

            # ---------------- query load / norm blocks ----------------
            qtiles = [None] * NQ_SH   # (qp0, qp1, q84) fp8 tiles per image

            def q_load(img):
                qp = [qlp.tile([128, 2, N], FP8, tag=f"qp{p}", name=f"qp{p}")
                      for p in range(2)]
                q4 = qlp.tile([128, N], FP8, tag="q4")
                for cb in range(CB):
                    st = qstgp.tile([128, N], F32, tag=f"qf{cb}")
                    di = nc.sync.dma_start(st[:],
                                           q_d[img, cb * 128:(cb + 1) * 128, :])
                    if last_sdma[0] is not None:
                        # don't let later query loads steal DMA bandwidth
                        # from the support prologue (shat gates all sims)
                        add_dep_helper(di.ins, last_sdma[0].ins, sync=True,
                                       reason="yield DMA bw to support load")
                    dst = qp[cb // 2][:, cb % 2, :] if cb < 4 else q4[:]
                    nc.scalar.copy(dst, st[:])
                qtiles[img] = (qp[0], qp[1], q4)

            simp_box = [None]
            qsums = [None] * NQ_SH
            ADD = mybir.AluOpType.add

            def q_sum(img):
                qp0, qp1, q4 = qtiles[img]
                slots = [qp0[:, 0, :], qp0[:, 1, :], qp1[:, 0, :], qp1[:, 1, :],
                         q4[:]]
                sqs = []
                for cb in range(CB):
                    sq = qsqp.tile([128, N], BF16, tag=f"qsq{cb}", name=f"qsq{cb}")
                    if cb < n_sq_act:
                        nc.scalar.square(sq[:], slots[cb])
                    else:
                        nc.vector.tensor_mul(sq[:], slots[cb], slots[cb])
                    sqs.append(sq)
                qsum = qsmp.tile([128, N], BF16, tag="qsum")
                adds = [(sqs[0], sqs[0], sqs[1]), (sqs[2], sqs[2], sqs[3]),
                        (sqs[0], sqs[0], sqs[2]), (qsum, sqs[0], sqs[4])]
                for k, (o, a, b) in enumerate(adds):
                    if k < n_gp_adds:
                        nc.gpsimd.tensor_tensor(o[:], a[:], b[:], ADD)
                    else:
                        nc.vector.tensor_add(o[:], a[:], b[:])
                qsums[img] = qsum

            def q_fin(img):
                qsum = qsums[img]
                qn_ps = simp_box[0].tile([1, M], F32, tag="ps", name="qn_ps")
                for h in range(2):
                    nc.tensor.matmul(qn_ps[:, h * 512:(h + 1) * 512], ones_b[:],
                                     qsum[:, h * 512:(h + 1) * 512],
                                     start=True, stop=True)
                # stash the (1, 1024) norm row permuted to j-major order so one
                # contiguous DMA scatters it into column layout
                qn_sb = stp.tile([1, N], F32, tag="stash")
                nc.vector.tensor_scalar_add(
                    qn_sb[:, :].rearrange("one (j nb) -> one j nb", j=128, nb=8),
                    qn_ps[0:1, :].rearrange("one (nb j) -> one j nb", nb=8, j=128),
                    0.0)
                nc.sync.dma_start(norms2cols[:, img * 8:(img + 1) * 8], qn_sb[:, :])

            # ---------------- sim matmuls + row max ----------------
            def sim_block(img):
                if img >= n_sim:
                    return
                qp0, qp1, q4 = qtiles[img]
                for nb in range(8):
                    ps = simp_box[0].tile([128, M], F32, tag="ps", name="ps")
                    if no_dr:
                        qps = [qp0, qp0, qp1, qp1, None]
                        for p in range(5):
                            for h in range(2):
                                hs = slice(h * 512, (h + 1) * 512)
                                ns = slice(nb * 128, (nb + 1) * 128)
                                if p < 4:
                                    l5 = qps[p][:, p % 2, ns]
                                    r5 = sp8[p // 2][:, p % 2, hs]
                                else:
                                    l5 = q4[:, ns]
                                    r5 = s84[:, hs]
                                nc.tensor.matmul(
                                    ps[:, hs], l5, r5,
                                    start=(p == 0), stop=(p == 4))
                    else:
                        lhs = [qp0[:, :, nb * 128:(nb + 1) * 128],
                               qp1[:, :, nb * 128:(nb + 1) * 128],
                               q4[:, nb * 128:(nb + 1) * 128]]
                        for p in range(3):
                            for h in range(2):
                                r = sp8[p][:, :, h * 512:(h + 1) * 512] if p < 2 \
                                    else s84[:, h * 512:(h + 1) * 512]
                                nc.tensor.matmul(
                                    ps[:, h * 512:(h + 1) * 512], lhs[p], r,
                                    start=(p == 0), stop=(p == 2),
                                    perf_mode=DR if p < 2 else None)
                    col = img * 8 + nb
                    nc.vector.reduce_max(maxcols[:, col:col + 1], ps[:, :],
                                         axis=AX_X)

            # prototype norm chain (per psum-half) -> shat in fp8
            pnorm = miscp.tile([1, M], F32, tag="pn")
            pn8 = miscp.tile([128, 8], F32, tag="pn8")
            pi8 = miscp.tile([128, 8], F32R, tag="pi8")
            pinv = miscp.tile([1, M], F32R, tag="pinv")
            bc_ps = pnps.tile([128, M], F32, tag="bc", name="bc_ps")
            bc_sb = miscp.tile([128, M], BF16, tag="bc_sb")

            def chain_h(h):
                hs = slice(h * 512, (h + 1) * 512)
                h4 = slice(h * 4, (h + 1) * 4)
                nc.scalar.sqrt(pnorm[:, hs], pn_ps[:, hs])
                nc.sync.dma_start(pn8[:, h4], pnorm[:, hs])
                with nc.allow_low_precision(reason="f32r is full 4-byte"):
                    nc.vector.reciprocal(pi8[:, h4], pn8[:, h4])
                nc.sync.dma_start(pinv[:, hs], pi8[:, h4])
                # broadcast partition 0 to all partitions via K=1 matmul
                nc.tensor.matmul(bc_ps[:, hs], ones_k1[:], pinv[:, hs],
                                 start=True, stop=True)
                nc.scalar.copy(bc_sb[:, hs], bc_ps[:, hs])
                for cb in range(CB):
                    dst = sp8[cb // 2][:, cb % 2, hs] if cb < 4 else s84[:, hs]
                    nc.vector.tensor_mul(dst, protos[cb][:, hs], bc_sb[:, hs])

            # ---------------- issue order ----------------
            for cb in range(4):
                s_block(cb, 0)
                s_block(cb, 1)
            # issue both s4 halves before the chain: chain_h(0)'s dependent
            # pn8/pinv DMA triggers would otherwise block s4h1's DMA triggers
            # in the sync-engine queue (head-of-line)
            s_block(4, 0)
            s_block(4, 1)
            chain_h(0)
            chain_h(1)
            # q0 is issued after the chain: its gated DMAs land during the
            # pnorm chain, and its ACT conversions must not head-of-line
            # block the chain's sqrt/squares in the scalar-engine queue
            q_load(0)
            pnps_ctx.__exit__(None, None, None)
            # sim psum ring opens only after the prologue psum frees its banks
            simp_ctx = tc.tile_pool(name="simps", bufs=simp_bufs, space="PSUM")
            simp_box[0] = simp_ctx.__enter__()

            q_load(1)
            q_sum(0)
            sim_block(0)
            for img in range(1, NQ_SH):
                q_sum(img)
                if img + 1 < NQ_SH:
                    q_load(img + 1)   # staging triggers ahead of the stash DMA
                q_fin(img - 1)
                if img == NQ_SH - 1:
                    q_fin(img)   # before the last sims so the tail is max-drain only
                sim_block(img)
            simp_ctx.__exit__(None, None, None)

            # ---------------- epilogue: scores ----------------
            with tc.tile_pool(name="ep", bufs=1) as ep, \
                 tc.tile_pool(name="eps", bufs=1, space="PSUM") as epp:
                qn = ep.tile([128, 128], F32, tag="qn")
                nc.scalar.sqrt(qn[:], norms2cols[:])
                qinv = ep.tile([128, 128], F32, tag="qinv")
                nc.vector.reciprocal(qinv[:], qn[:])
                scaled = ep.tile([128, 128], F32, tag="scaled")
                nc.vector.scalar_tensor_tensor(
                    out=scaled[:], in0=maxcols[:], scalar=1.0 / float(N),
                    in1=qinv[:], op0=MULT, op1=MULT)
                fin_ps = epp.tile([1, 128], F32, tag="fin")
                nc.tensor.matmul(fin_ps[:], ones_f[:], scaled[:],
                                 start=True, stop=True)
                fin_sb = ep.tile([1, 128], F32, tag="finsb")
                nc.scalar.copy(fin_sb[:], fin_ps[:])
                scores = ep.tile([1, NQ_SH], F32, tag="scores")
                nc.vector.reduce_sum(
                    scores[:],
                    fin_sb[:].rearrange("p (i b) -> p i b", i=16, b=8),
                    axis=AX_X)
                nc.sync.dma_start(out_d[:, :], scores[:])

    nc.compile()
    return nc


_NC_CACHE = None


def _get_nc():
    global _NC_CACHE
    if _NC_CACHE is None:
        _NC_CACHE = build_bass()
    return _NC_CACHE


def kernel(query_features: np.ndarray, support_features: np.ndarray) -> np.ndarray:
    NQ = query_features.shape[0]
    assert NQ == NQ_SH * NCORES
    q = np.ascontiguousarray(
        query_features.reshape(NQ, CB * 128, N).astype(np.float32, copy=False))
    s = np.ascontiguousarray(
        support_features.reshape(5, CB * 128, M).astype(np.float32, copy=False))
    nc = _get_nc()
    in_maps = [
        {"q": q[i * NQ_SH:(i + 1) * NQ_SH], "s": s} for i in range(NCORES)
    ]
    res = bass_utils.run_bass_kernel_spmd(nc, in_maps, core_ids=list(range(NCORES)))
    out = np.concatenate(
        [np.asarray(res.results[i]["out"]).reshape(NQ_SH) for i in range(NCORES)])
    return out.astype(np.float32, copy=False)
